# revision 31
# baseline (speedup 1.0000x reference)
"""LocalIsing energy kernel for Trainium2 (8 NeuronCores, data-parallel over batch).

reference:  energy[b] = x[b] @ J1 + sum_c J2[c] * x[b, p0[c]] * x[b, p1[c]]

The pair term is a quadratic form: scatter-add J2 into W[512,512] at (p0,p1)
(host-side, cheap: 130816 elements), then
    energy[b] = sum_j x[b,j] * (x @ W)[b,j]  +  x[b] @ J1
Each core handles 128 batch rows.

Since x_i*W_ij*x_j is symmetric in (i,j), any lower-triangle mass is folded
into the upper triangle host-side (Wu = triu(W+W.T,1) + diag(W)), so W is
strictly upper-triangular: K-tile k only has nonzero columns [128k, 512).
Packing just those ranges cuts the W stream from 512KB to 320KB and the
PE moving-operand columns from 2048 to 1280.

All inputs are fp16 (x is exactly representable; W/J1 are pre-scaled by 2^16
so ~1e-4 couplings stay out of fp16's subnormal range, and the fused reduce
rescales by 2^-16 in its scalar stage for free — rel_l2 ~2e-4 vs 1.6e-3 for
bf16 at identical byte count and PE speed).

Streams (fp16, ~577KB/core vs 1.84MB fp32 baseline):
  mega1 [128, 1024]: xt k-tiles (4x128) || w0 (512)      scalar HWDGE
  mega1b[128,  640]: w1 (384) || w2 (256)                scalar HWDGE
  mega2 [128,  640]: w3 (128) || xs (512)                sync HWDGE
  jo    [1, 640]:    J1 (512) || ones (128)              sync HWDGE
Each matmul is gated only on the DMA chunk it needs, so the PE chases the
incoming stream.  J1 enters the PSUM accumulation as a rank-1 matmul
(ones x J1) instead of a 256KB broadcast.  The energy column [128,1] is
repacked via the DVE 32x32 block transpose so the output DMA is 4 x 128B
packets instead of 128 x 4B packets (whose completion costs ~9us).
"""

import numpy as np
from contextlib import ExitStack

import concourse.tile as tile
from concourse import bacc, mybir
from concourse.bass_utils import run_bass_kernel_spmd

N = 512          # spins
B = 1024         # batch
NCORES = 8
BS = B // NCORES  # 128 rows per core = one partition tile
KT = N // 128     # 4 contraction tiles
CA = 1024         # mega1 cols:  xt(512) | w0(512)
CB = 640          # mega1b cols: w1(384) | w2(256)
CC = 640          # mega2 cols:  w3(128) | xs(512)

_cached_nc = None


SC = 2.0 ** 16   # fp16 scale: J-values ~1e-4 sit in fp16's subnormal range;
                 # scaling up (exact power of 2) keeps them normal, and the
                 # fused reduce rescales by 1/SC in its scalar stage for free.


def _build():
    f32 = mybir.dt.float32
    f16 = mybir.dt.float16
    nc = bacc.Bacc(
        "TRN2", target_bir_lowering=False, debug=False, num_devices=1
    )
    mega1 = nc.dram_tensor("mega1", [BS, CA], f16, kind="ExternalInput")
    mega1b = nc.dram_tensor("mega1b", [BS, CB], f16, kind="ExternalInput")
    mega2 = nc.dram_tensor("mega2", [BS, CC], f16, kind="ExternalInput")
    jo = nc.dram_tensor("jo", [1, N + BS], f16, kind="ExternalInput")
    en = nc.dram_tensor("energy", [4, 32], f32, kind="ExternalOutput")

    with tile.TileContext(nc) as tc, ExitStack() as ctx:
        sb = ctx.enter_context(tc.tile_pool(name="sb", bufs=1))
        ps = ctx.enter_context(tc.tile_pool(name="ps", bufs=1, space="PSUM"))

        jo_sb = sb.tile([1, N + BS], f16)
        m1a = sb.tile([BS, CA], f16)
        m1b = sb.tile([BS, CB], f16)
        m2 = sb.tile([BS, CC], f16)
        nc.scalar.dma_start(m1a, mega1[:, :])
        nc.scalar.dma_start(m1b, mega1b[:, :])
        nc.sync.dma_start(m2, mega2[:, :])
        nc.sync.dma_start(jo_sb, jo[:, :])

        # staging block for the 32x32 transpose; column 0 = energy
        ecol = sb.tile([BS, 32], f32)
        nc.gpsimd.memset(ecol, 0.0)

        # y[b,j] = J1[j] + sum_k x[b,k] W[k,j]   (5 accumulating matmuls;
        # tile k of the strictly-upper-tri W only writes y[:, 128k:])
        y = ps.tile([BS, N], f32)
        nc.tensor.matmul(
            y, jo_sb[:1, N:], jo_sb[:1, :N], start=True, stop=False
        )
        w_tiles = [
            (m1a[:, N:], 0),          # w0: cols [0, 512)
            (m1b[:, : 3 * 128], 128), # w1: cols [128, 512)
            (m1b[:, 3 * 128 :], 256), # w2: cols [256, 512)
            (m2[:, :128], 384),       # w3: cols [384, 512)
        ]
        for k, (w, c0) in enumerate(w_tiles):
            nc.tensor.matmul(
                y[:, c0:],
                m1a[:, k * 128 : (k + 1) * 128],
                w,
                start=False,
                stop=(k == KT - 1),
            )

        # e[b] = sum_j y[b,j] * x[b,j]  (fused multiply + row-sum on DVE)
        xs = m2[:, 128:]
        scr = sb.tile([BS, N], f32)
        nc.vector.scalar_tensor_tensor(
            out=scr,
            in0=y,
            scalar=1.0 / SC,
            in1=xs,
            op0=mybir.AluOpType.mult,
            op1=mybir.AluOpType.mult,
            accum_out=ecol[:, 0:1],
        )

        # 32x32 block transpose: row 32*g of `et` holds e[32g : 32g+32],
        # so the output DMA is 4 contiguous 128B packets instead of 128x4B.
        et = sb.tile([BS, 32], f32)
        nc.vector.transpose(et, ecol)
        nc.sync.dma_start(en[:, :], et[0:BS:32, :])
    nc.finalize()
    return nc


def _pack_inputs(x, J1, J2, pairs):
    x = np.asarray(x, dtype=np.float32)
    J1 = np.asarray(J1, dtype=np.float32)
    J2f = np.asarray(J2, dtype=np.float64)
    pairs = np.asarray(pairs)
    f16 = mybir.dt.np(mybir.dt.float16)

    # Scatter-add J2 into W (handles duplicate pairs exactly like the
    # reference's gather-sum), then fold the (symmetric) quadratic form
    # into a strictly-upper-triangular matrix.
    idx = pairs[:, 0].astype(np.int64) * N + pairs[:, 1].astype(np.int64)
    W = np.bincount(idx, weights=J2f, minlength=N * N).reshape(N, N)
    Wu = (np.triu(W + W.T, 1) + np.diag(np.diag(W))) * SC
    # w_k[p, :] = Wu[128k + p, 128k:]  (the nonzero columns of row 128k+p)
    wk = [
        np.ascontiguousarray(Wu[k * 128 : (k + 1) * 128, k * 128 :]).astype(f16)
        for k in range(KT)
    ]
    jo = np.concatenate([J1 * SC, np.ones(BS, np.float32)]).astype(f16)[None, :]

    in_maps = []
    for c in range(NCORES):
        xs = x[c * BS : (c + 1) * BS]                      # [128, 512]
        # xtp[p, k*128+b] = x[c*128+b, k*128+p]
        xtp = np.ascontiguousarray(
            xs.T.reshape(KT, 128, BS).transpose(1, 0, 2).reshape(128, KT * BS)
        ).astype(f16)
        in_maps.append(
            {
                "mega1": np.ascontiguousarray(np.concatenate([xtp, wk[0]], axis=1)),
                "mega1b": np.ascontiguousarray(np.concatenate([wk[1], wk[2]], axis=1)),
                "mega2": np.ascontiguousarray(
                    np.concatenate([wk[3], xs.astype(f16)], axis=1)
                ),
                "jo": jo,
            }
        )
    return in_maps


def kernel(x, J1, J2, pairs):
    global _cached_nc
    if _cached_nc is None:
        _cached_nc = _build()
    in_maps = _pack_inputs(x, J1, J2, pairs)
    res = run_bass_kernel_spmd(_cached_nc, in_maps, core_ids=list(range(NCORES)))
    return np.concatenate([r["energy"].reshape(-1) for r in res.results])

